# revision 5
# baseline (speedup 1.0000x reference)
"""Trainium2 Bass kernel for EfficientViT-style attention block.

Reference computation (per batch element b of 16):
    x: [256, 1024]  (C=256 channels, N=32*32 spatial)
    q = (sq*wq) @ x + bq        -> [128, N]  (8 heads x 16 key dims)
    k = (sk*wk) @ x + bk        -> [128, N]
    v = (sv*wv) @ x + bv        -> [256, N]  (8 heads x 32 v dims)
    per head: attn = softmax(q_h^T k_h, axis=-1); o_h = v_h @ attn^T
    out = (sp*wp) @ relu(concat o_h) + bp

Sharding: data-parallel over batch: 8 cores x 2 batch elements.

v2 kernel strategy per core (all matmuls bf16 inputs, fp32 PSUM):
- softmax exp split between ScalarE (ACT) and VectorE (DVE): per m-tile,
  heads 0-1 exp on ACT; heads 2-3 on ACT or on DVE (custom fused ops:
  deg-4 poly of exp(x/16) then ^16 by squaring) per DVE_MTS schedule.
- v-projection bias bv folded into vt (so attention output needs no
  later bias); normalize+relu fused into ONE custom DVE op using a
  bitwise-NOT-seeded 1-step-NR reciprocal.
- AV + denominator matmuls issued as 4-way column-packed quads
  (4 concurrent 32-col strips); redundant ones-LDWEIGHTS deleted by a
  post-pass exploiting PE weight residency (HW-verified).
- scores: 4-way row-packed per m-tile (as v1).
"""

import numpy as np
import ml_dtypes

B, C, H, W = 16, 256, 32, 32
N = H * W            # 1024
NH, KD, DV = 8, 16, 32
NB = 2               # batch elements per core
NCORES = 8
P = 128
NT = 512             # n-tile (psum bank)

BF16 = ml_dtypes.bfloat16

# deg-4 poly p(x) ~= exp(x/16) on [-14, 14], p(0)=1; e = p^16
EA1, EA2, EA3, EA4 = 6.24307520e-02, 1.95715106e-03, 4.24595854e-05, 6.23134112e-07
# 1-NR reciprocal constants (bitwise-NOT seed)
RC0, RC1 = -0.235497936615358, 2.0017323488807213

# m-tiles whose heads 2-3 exp runs on the DVE instead of ScalarE
DVE_MTS = (1, 3, 5, 7)
# q/k proj bias-copies routed to ScalarE (piece index mod): 0 = none
QK_ON_ACT = 0

_CACHE = {}


def _register_dve_ops():
    import concourse.dve_ops as D
    from concourse.dve_spec import (
        Spec, Src0, Src1, C0, C1, C2, Zero, One, lower, AluOp, Bin,
        maxx, sq, _spill_c3_to_src1, _has_src1,
    )
    from concourse.dve_uop import DveOpSpec

    def mk(name, body, ref):
        if name in D._SUB_OPCODE_FOR_NAME:
            return next(o for o in D.OPS if o.name == name)
        spec = Spec(body=body, reference=ref)
        row = D._CUSTOM_DVE_ROW_BASE + len(D.OPS)
        uops = lower(spec, ver="v3")
        sha = DveOpSpec(name=name, opcode=row, uops=uops,
                        rd1_en=_has_src1(spec)).sha("v3")
        op = D.DveOp(name, spec, subdim=False, uops_sha={"v3": sha})
        D.OPS.append(op)
        D.CUSTOM_DVE_SPECS[name] = spec
        D._SUB_OPCODE_FOR_NAME[name] = row
        return op

    from concourse.dve_spec import C3
    x = Src0
    body1 = ((((x * C0) + C1) * x + C2) * x + C3) * x + One

    def exp1_ref(in0, in1, s0, s1, imm2):
        return ((((in0 * s0) + s1) * in0 + imm2) * in0 + in1[:, 0:1]) * in0 + 1.0

    EXP1 = mk("ANT_EXP16_P1", _spill_c3_to_src1(body1), exp1_ref)
    EXP2 = mk("ANT_EXP16_P2", sq(sq(sq(sq(Src0)))),
              lambda in0, in1, s0, s1, imm2: (((in0**2)**2)**2)**2)

    body3 = maxx(Src0 * Src1, Zero)
    NORM = mk("ANT_RELU_MUL", body3,
              lambda in0, in1, s0, s1, imm2: np.maximum(in0 * in1, 0))
    return EXP1, EXP2, NORM


def _dedup_ldweights(nc):
    """Delete InstLdweights that reload weights already resident in the PE
    array (identical source AP + position, no intervening clobber of the
    covered 32-column strips). HW-verified: a matmul whose LDW is removed
    uses the resident weights."""
    ndel = 0
    for blk in nc.m.functions[0].blocks:
        insts = list(blk.instructions)
        resident = {}           # col strip -> ldw identity
        clobber_seq = 0
        keep = []
        for ins in insts:
            if type(ins).__name__ != 'InstLdweights':
                keep.append(ins)
                continue
            w = ins.ins[0]
            ap = [list(d) for d in w.ap]
            pnum = ap[0][1]
            fnum = ap[-1][1] if len(ap) > 1 else 1
            tp = ins.tile_position or (0, 0)
            ident = (w.memref, w.offset, str(ap), tp)
            c0 = tp[1] // 32
            nstrips = (fnum + 31) // 32
            strips = range(c0, min(c0 + nstrips, 4))
            full_rows = (tp[0] == 0 and pnum == P)
            if full_rows and all(resident.get(s) == ident for s in strips):
                ndel += 1
                continue
            if full_rows:
                for s in strips:
                    resident[s] = ident
            else:
                clobber_seq += 1
                for s in strips:
                    resident[s] = ("partial", clobber_seq)
            keep.append(ins)
        if len(keep) != len(insts):
            blk.instructions = keep
    return ndel


def _build_nc():
    import concourse.tile as tile
    from concourse import bacc, mybir

    f32 = mybir.dt.float32
    bf16 = mybir.dt.bfloat16
    Alu = mybir.AluOpType
    Act = mybir.ActivationFunctionType

    EXP1, EXP2, NORM = _register_dve_ops()

    nc = bacc.Bacc()

    xb = nc.declare_dram_parameter("xb", [NB, C, N], bf16, isOutput=False)
    # all weights in one tensor: [tile, c, {wq|wk|wv|wp} x 256]
    wall = nc.declare_dram_parameter("wall", [2, P, 4 * 256], bf16,
                                     isOutput=False)
    # per-partition bias vectors: [partition, group, {q,k,p}]
    biases = nc.declare_dram_parameter("biases", [P, 2, 3], f32, isOutput=False)
    # v bias broadcast along free dim (dh layout), twice for mt-pair copies
    bvrow = nc.declare_dram_parameter("bvrow", [P, 512], f32, isOutput=False)
    out = nc.declare_dram_parameter("out", [NB, C, N], f32, isOutput=True)

    with tile.TileContext(nc) as tc:
        with (
            tc.tile_pool(name="consts", bufs=1) as consts,
            tc.tile_pool(name="xp", bufs=2) as xp,
            tc.tile_pool(name="qk", bufs=2) as qk,
            tc.tile_pool(name="vtp", bufs=2) as vtp,
            tc.tile_pool(name="ep", bufs=2) as ep,
            tc.tile_pool(name="etp", bufs=2) as etp,
            tc.tile_pool(name="rp", bufs=2) as rp,
            tc.tile_pool(name="yp", bufs=4) as yp,
            tc.tile_pool(name="ps_s", bufs=1, space="PSUM") as ps_s,
            tc.tile_pool(name="ps_av", bufs=1, space="PSUM") as ps_av,
            tc.tile_pool(name="ps_den", bufs=1, space="PSUM") as ps_den,
            tc.tile_pool(name="ps_proj", bufs=2, space="PSUM") as ps_proj,
        ):
            # --- memsets + PE warmup first (no DMA deps): HAM ramps while
            # the input DMAs are in flight ---
            ones_sb = consts.tile([P, DV], bf16, tag="ones")
            nc.vector.memset(ones_sb[:], 1.0)
            warm_rhs = consts.tile([P, 256], bf16, tag="warm_rhs")
            nc.vector.memset(warm_rhs[:], 0.0)
            c3t = consts.tile([P, 1], f32, tag="c3")
            nc.vector.memset(c3t[:], EA1)
            for wi in range(6):
                wps = ps_proj.tile([P, NT], f32, tag="proj",
                                   name=f"warm{wi}")
                nc.tensor.matmul(wps[0:DV, 0:256], lhsT=ones_sb[:],
                                 rhs=warm_rhs[:], start=True, stop=True)

            # --- constants + x into SBUF; weights early (gate first projs) ---
            x_tiles = []
            for b in range(NB):
                x_tiles.append(xp.tile([P, 2, N], bf16, tag="x",
                                       name=f"x{b}"))
            w_sb = consts.tile([P, 2, 4 * 256], bf16, tag="w")
            nc.sync.dma_start(out=x_tiles[0][:, 0, :], in_=xb[0, 0:P, :])
            nc.sync.dma_start(out=w_sb[:, 0, :], in_=wall[0])
            nc.sync.dma_start(out=x_tiles[0][:, 1, :], in_=xb[0, P:2 * P, :])
            nc.sync.dma_start(out=w_sb[:, 1, :], in_=wall[1])
            bias_sb = consts.tile([P, 2, 3], f32, tag="bias")
            nc.sync.dma_start(out=bias_sb[:], in_=biases[:])
            bvrow_sb = consts.tile([P, 512], f32, tag="bvrow")
            nc.sync.dma_start(out=bvrow_sb[:], in_=bvrow[:])
            for ct in range(2):
                nc.gpsimd.dma_start(out=x_tiles[1][:, ct, :],
                                    in_=xb[1, ct * P:(ct + 1) * P, :])
            # touch ops: bring DVE/ACT clocks past the constant DMAs so
            # downstream 1-wait-limited instructions only wait on PE; the
            # ACT touch also pre-loads the exp table set.
            scratch = consts.tile([P, 2], f32, tag="scratch")
            nc.vector.tensor_copy(out=scratch[:, 0:1], in_=bias_sb[:, 0, 0:1])
            nc.scalar.activation(out=scratch[:, 1:2], in_=bias_sb[:, 0, 1:2],
                                 func=Act.Exp)

            def bias_ap(kind, g):
                i = {"q": 0, "k": 1, "p": 2}[kind]
                return bias_sb[:, g, i:i + 1]

            # ---------- per-b building blocks ----------
            qkv = {}      # b -> dict(x=, q=, k=, vt=)
            r_tiles = {}  # b -> r_sb
            piece_ctr = [0]

            def qkv_piece(b, kind, g, arg):
                x_sb = qkv[b]["x"]
                if kind in ("q", "k"):
                    woff = 0 if kind == "q" else 256
                    dst = qkv[b][kind]
                    nt = arg
                    ps = ps_proj.tile([P, NT], f32, tag="proj",
                                      name=f"pp_{b}{kind}{g}{nt}")
                    for ct in range(2):
                        nc.tensor.matmul(
                            ps[:],
                            lhsT=w_sb[:, ct, woff + P * g:woff + P * (g + 1)],
                            rhs=x_sb[:, ct, nt * NT:(nt + 1) * NT],
                            start=(ct == 0), stop=(ct == 1))
                    piece_ctr[0] += 1
                    o = dst[:, g, nt * NT:(nt + 1) * NT]
                    if QK_ON_ACT and piece_ctr[0] % QK_ON_ACT == 0:
                        nc.scalar.activation(out=o, in_=ps[:],
                                             func=Act.Copy,
                                             bias=bias_ap(kind, g))
                    else:
                        nc.vector.tensor_scalar_add(
                            out=o, in0=ps[:], scalar1=bias_ap(kind, g))
                else:
                    mt0 = arg
                    ps = ps_proj.tile([P, 2, 256], f32, tag="proj",
                                      name=f"pv_{b}{mt0}")
                    for i in range(2):
                        for ct in range(2):
                            nc.tensor.matmul(
                                ps[:, i, :],
                                lhsT=x_sb[:, ct, (mt0 + i) * P:(mt0 + i + 1) * P],
                                rhs=w_sb[:, ct, 512:768],
                                start=(ct == 0), stop=(ct == 1))
                    # fold v bias: vt = v + bv  (bias rides the AV matmul)
                    nc.vector.scalar_tensor_tensor(
                        out=qkv[b]["vt"][:, mt0:mt0 + 2, :],
                        in0=ps[:], scalar=1.0, in1=bvrow_sb[:],
                        op0=Alu.bypass, op1=Alu.add)

            def emit_qkv_head(b):
                """Allocate b's tiles + minimum pieces for its first scores:
                k(g0, both nt) and q(g0, nt0). Returns deferred closures."""
                qkv[b] = dict(
                    x=x_tiles[b],
                    q=qk.tile([P, 2, N], bf16, tag="q", name=f"q{b}"),
                    k=qk.tile([P, 2, N], bf16, tag="k", name=f"k{b}"),
                    vt=vtp.tile([P, 8, 256], bf16, tag="vt", name=f"vt{b}"))
                r_tiles[b] = rp.tile([P, 2, N], bf16, tag="r", name=f"r{b}")
                for kind, g, i in (("k", 0, 0), ("k", 0, 1), ("q", 0, 0)):
                    qkv_piece(b, kind, g, i)
                rest = [("q", 0, 1), ("vt", 0, 0), ("vt", 0, 2),
                        ("vt", 0, 4), ("vt", 0, 6),
                        ("q", 1, 0), ("k", 1, 0), ("k", 1, 1), ("q", 1, 1)]
                return [lambda kind=kind, g=g, i=i: qkv_piece(b, kind, g, i)
                        for kind, g, i in rest]

            def avden_chunks(pend):
                """Pending iteration's AV + denominator as 8 chunks of 2
                quads. Each quad = 4 col-strip-packed MMs (2 av + 2 den)."""
                b, g, nt = pend["key"]
                av, den, e_all = pend["av"], pend["den"], pend["e"]
                vt_sb = qkv[b]["vt"]

                def quad(mt, phase):
                    st, sp = (mt == 0), (mt == 7)
                    for p in range(4):
                        if (p % 2 == 0) == (phase == 0):
                            h = 4 * g + p
                            nc.tensor.matmul(
                                av[32 * p:32 * p + 32, :],
                                lhsT=vt_sb[:, mt, 32 * h:32 * h + 32],
                                rhs=e_all[:, mt, p * NT:(p + 1) * NT],
                                start=st, stop=sp, tile_position=(0, 32 * p))
                        else:
                            nc.tensor.matmul(
                                den[32 * p:32 * p + 32, 0, :],
                                lhsT=ones_sb[:],
                                rhs=e_all[:, mt, p * NT:(p + 1) * NT],
                                start=st, stop=sp, tile_position=(0, 32 * p))

                chunks = []
                for phase in range(2):
                    for mt0 in range(0, 8, 2):
                        def chunk(mt0=mt0, phase=phase):
                            quad(mt0, phase)
                            quad(mt0 + 1, phase)
                        chunks.append(chunk)
                return chunks

            def emit_finalize(pend):
                """Normalize+relu: 2-NR reciprocal (PSUM->SBUF), then fused
                relu(av*recip); if it closes a batch element, also emit the
                output projection."""
                b, g, nt = pend["key"]
                av, den = pend["av"], pend["den"]
                recip_t = etp.tile([P, 1, NT], f32, tag="rcp")
                nc.vector.reciprocal_approx_fast(out=recip_t[:, 0, :],
                                                 in_=den[:, 0, :])
                nc.vector._custom_dve(
                    NORM, out=r_tiles[b][:, g, nt * NT:(nt + 1) * NT],
                    in0=av[:], in1=recip_t[:])
                if (g, nt) == (1, 1):
                    r_sb = r_tiles[b]
                    for ct in range(2):
                        y_sb = yp.tile([P, N], f32, tag="y")
                        for nt2 in range(2):
                            ps = ps_proj.tile([P, NT], f32, tag="proj")
                            for gg in range(2):
                                nc.tensor.matmul(
                                    ps[:],
                                    lhsT=w_sb[:, gg, 768 + ct * P:768 + (ct + 1) * P],
                                    rhs=r_sb[:, gg, nt2 * NT:(nt2 + 1) * NT],
                                    start=(gg == 0), stop=(gg == 1))
                            nc.vector.tensor_scalar_add(
                                out=y_sb[:, nt2 * NT:(nt2 + 1) * NT],
                                in0=ps[:], scalar1=bias_ap("p", ct))
                            nc.sync.dma_start(
                                out=out[b, ct * P:(ct + 1) * P,
                                        nt2 * NT:(nt2 + 1) * NT],
                                in_=y_sb[:, nt2 * NT:(nt2 + 1) * NT])

            # ---------- software-pipelined main loop ----------
            pending = None
            qkv_queue = []
            for b in range(NB):
                for g in range(2):
                    for nt in range(2):
                        drain_all = (b, g, nt) == (0, 0, 0)
                        if drain_all:
                            qkv_queue.extend(emit_qkv_head(0))
                        q_sb, k_sb = qkv[b]["q"], qkv[b]["k"]
                        av = ps_av.tile([P, NT], f32, tag="av")
                        den = ps_den.tile([P, 1, NT], f32, tag="den")
                        e_all = ep.tile([P, 8, 4 * NT], bf16, tag="e")
                        chunks = avden_chunks(pending) if pending else []
                        ci = 0
                        for mt in range(8):
                            sts = [ps_s.tile([P, 2 * NT], f32, tag=t,
                                             name=f"s_{b}{g}{nt}{mt}{t}")
                                   for t in ("sa", "sb")]
                            for j in range(4):
                                row = 32 * j
                                nc.tensor.matmul(
                                    sts[j // 2][:, (j % 2) * NT:
                                                (j % 2 + 1) * NT],
                                    lhsT=k_sb[row:row + KD, g,
                                              mt * P:(mt + 1) * P],
                                    rhs=q_sb[row:row + KD, g,
                                             nt * NT:(nt + 1) * NT],
                                    start=True, stop=True,
                                    tile_position=(row, 0))
                            # heads 0-1 exp always on ScalarE
                            nc.scalar.activation(
                                out=e_all[:, mt, 0:2 * NT],
                                in_=sts[0][:], func=Act.Exp)
                            if ci < len(chunks):
                                chunks[ci]()
                                ci += 1
                            # heads 2-3: DVE or ScalarE
                            if mt in DVE_MTS:
                                et = etp.tile([P, 2 * NT], f32, tag="et")
                                nc.vector._custom_dve(
                                    EXP1, out=et[:], in0=sts[1][:],
                                    in1=c3t[:], s0=EA4, s1=EA3, imm2=EA2)
                                nc.vector._custom_dve(
                                    EXP2, out=e_all[:, mt, 2 * NT:4 * NT],
                                    in0=et[:])
                            else:
                                nc.scalar.activation(
                                    out=e_all[:, mt, 2 * NT:4 * NT],
                                    in_=sts[1][:], func=Act.Exp)
                            if qkv_queue and (drain_all or mt % 2 == 1):
                                qkv_queue.pop(0)()
                                if drain_all and qkv_queue:
                                    qkv_queue.pop(0)()
                        while ci < len(chunks):
                            chunks[ci]()
                            ci += 1
                        if pending:
                            emit_finalize(pending)
                        if drain_all and NB > 1:
                            qkv_queue.extend(emit_qkv_head(1))
                        pending = dict(key=(b, g, nt), av=av, den=den,
                                       e=e_all)
            # drain the last iteration
            for chunk in avden_chunks(pending):
                chunk()
            emit_finalize(pending)

    ndel = _dedup_ldweights(nc)
    if not nc.is_finalized():
        nc.finalize()
    nc._ant_ldw_deleted = ndel
    return nc


def _prep_consts(wq, sq, bq, wk, sk, bk, wv, sv, bv, wp, sp, bp):
    """Host-side weight prep. Returns dict of per-core-identical arrays."""
    wq_s = (sq[:, None] * wq).astype(np.float32)
    wk_s = (sk[:, None] * wk).astype(np.float32)
    wv_s = (sv[:, None] * wv).astype(np.float32)
    wp_s = (sp[:, None] * wp).astype(np.float32)

    def pad_qk(w_s, bias):
        wT_pad = np.zeros((256, 256), np.float32)   # [c, gcol]
        b_pad = np.zeros(256, np.float32)
        for g in range(2):
            for j in range(4):
                h = 4 * g + j
                col = 128 * g + 32 * j
                wT_pad[:, col:col + KD] = w_s[KD * h:KD * (h + 1), :].T
                b_pad[col:col + KD] = bias[KD * h:KD * (h + 1)]
        return (wT_pad.reshape(2, P, 256).astype(BF16),
                b_pad.reshape(2, P, 1).astype(np.float32))

    wqT, bqp = pad_qk(wq_s, bq)
    wkT, bkp = pad_qk(wk_s, bk)
    wvT = wv_s.T.copy().reshape(2, P, 256).astype(BF16)   # [c, dh]
    wpT = wp_s.T.copy().reshape(2, P, 256).astype(BF16)   # [dh, c]
    wall = np.concatenate([wqT, wkT, wvT, wpT], axis=2)   # [2, 128, 1024]
    bpp = bp.reshape(2, P).astype(np.float32)
    biases = np.zeros((P, 2, 3), np.float32)
    for g in range(2):
        biases[:, g, 0] = bqp[g, :, 0]
        biases[:, g, 1] = bkp[g, :, 0]
        biases[:, g, 2] = bpp[g]
    bvrow = np.broadcast_to(
        np.concatenate([bv, bv]).astype(np.float32)[None, :],
        (P, 512)).copy()
    return dict(wall=wall, biases=biases, bvrow=bvrow)


def make_in_maps(inputs):
    x = np.ascontiguousarray(inputs["x"]).reshape(B, C, N).astype(BF16)
    consts = _prep_consts(*[np.asarray(inputs[k], np.float32) for k in
                            ["wq", "sq", "bq", "wk", "sk", "bk",
                             "wv", "sv", "bv", "wp", "sp", "bp"]])
    in_maps = []
    for core in range(NCORES):
        m = dict(consts)
        m["xb"] = np.ascontiguousarray(x[NB * core:NB * (core + 1)])
        in_maps.append(m)
    return in_maps


def gather_out(results):
    parts = [np.asarray(results[i]["out"], np.float32) for i in range(NCORES)]
    return np.concatenate(parts, axis=0).reshape(B, C, H, W)


def get_nc():
    if "nc" not in _CACHE:
        _CACHE["nc"] = _build_nc()
    return _CACHE["nc"]


def kernel(**inputs):
    import os
    os.environ.setdefault("BASS_NEVER_TRACE", "1")
    from concourse.bass_utils import run_bass_kernel_spmd
    nc = get_nc()
    in_maps = make_in_maps(inputs)
    res = run_bass_kernel_spmd(nc, in_maps, core_ids=list(range(NCORES)),
                               trace=False)
    return gather_out(res.results)


if __name__ == "__main__":
    nc = _build_nc()
    print("built ok; deleted LDWs:", nc._ant_ldw_deleted)
